# revision 1
# baseline (speedup 1.0000x reference)
"""Self-contained Trainium2 Bass kernel for the CharRNN problem:
2-layer LSTM (B=32, T=256, H=256) + V=32000 softmax cross-entropy mean loss.

Strategy (8 NeuronCores, SPMD):
  * the LSTM recurrence is replicated on every core (it is latency-bound, so
    batch-sharding would not make it faster and would need collectives)
  * the dominant softmax matmul + exp is sharded over the vocab: each core
    owns a 4000-wide shard of softmax_w, computes logits for all 8192 rows
    against its shard, and reduces them to per-row sum(exp(logit)) plus the
    per-row target logit (rows whose target falls in the shard)
  * the host combines: loss_r = log(sum_cores se_r) - tgt_logit_r

Device-side layout (per core):
  * rows are TIME-MAJOR: r = t*B + b, so a 128-row tile = 4 timesteps
  * xs^T / hs^T activation slabs [128, 8192] bf16, hidden dim on partitions
  * LSTM gates z: psum [32, 1024] (batch on partitions), gate columns
    permuted to [i, o, f, j] on the host so one sigmoid covers [i, o]
  * h is transposed back to hidden-major each step with PE-transposes
  * target logits: gather softmax_w columns by target id (gpsimd ap_gather
    over an int16-pair view), multiply with hs^T, reduce with a ones-vector
    matmul
"""
import os
import numpy as np
import ml_dtypes
import concourse.bass as bass
import concourse.mybir as mybir
import concourse.tile as tile
from concourse import bacc
from concourse.masks import make_identity
from concourse.bass_utils import run_bass_kernel_spmd

F32 = mybir.dt.float32
BF16 = mybir.dt.bfloat16
I32 = mybir.dt.int32
I16 = mybir.dt.int16
AF = mybir.ActivationFunctionType
ALU = mybir.AluOpType

B, T, H, V, NCORES = 32, 256, 256, 32000, 8


def build_charrnn(T=256, V=32000, n_cores=8, has_b1=False, has_b2=False,
                  has_swb=False, num_devices=8):
    B, H = 32, 256
    G4 = 4 * H                      # 1024 gate width
    VS = V // n_cores               # vocab shard per core
    BT = B * T
    RT = BT // 128                  # 128-row tiles (4 steps each)
    assert T % 4 == 0 and BT % 128 == 0

    # vocab chunking for the exp pass: one psum BANK per chunk — a matmul
    # may not cross a psum bank boundary (HW corrupts accumulation if the
    # write spans banks; sim does not model this)
    CH = max(d for d in range(1, 513) if VS % d == 0)
    NCHUNK = VS // CH

    nc = bacc.Bacc("TRN2", target_bir_lowering=False, debug=False,
                   num_devices=num_devices)

    # ---------------- DRAM I/O ----------------
    ids_d = nc.dram_tensor("ids", (RT, 128, 1), I32, kind="ExternalInput")
    emb_d = nc.dram_tensor("emb", (V, H), F32, kind="ExternalInput")
    w1_d = nc.dram_tensor("w1", (4, 128, G4), BF16, kind="ExternalInput")
    w2_d = nc.dram_tensor("w2", (4, 128, G4), BF16, kind="ExternalInput")
    sw_d = nc.dram_tensor("sw", (2, 128, VS), BF16, kind="ExternalInput")
    swp_d = nc.dram_tensor("swp", (2, 128, VS, 2), I16, kind="ExternalInput")
    tgi_d = nc.dram_tensor("tgi", (RT, 128, 8), I16, kind="ExternalInput")
    if has_b1:
        b1_d = nc.dram_tensor("b1p", (32, G4), F32, kind="ExternalInput")
    if has_b2:
        b2_d = nc.dram_tensor("b2p", (32, G4), F32, kind="ExternalInput")
    if has_swb:
        swb_d = nc.dram_tensor("swbp", (128, VS), F32, kind="ExternalInput")
    se_d = nc.dram_tensor("se_out", (128, RT * NCHUNK), F32,
                          kind="ExternalOutput")
    tg_d = nc.dram_tensor("tg_out", (1, BT), F32, kind="ExternalOutput")

    with tile.TileContext(nc) as tc:
        with tc.tile_pool(name="persist", bufs=1) as pp:
            # ---- persistent SBUF ----
            w1_sb = pp.tile([128, 4, G4], BF16, tag="w1")
            w2_sb = pp.tile([128, 4, G4], BF16, tag="w2")
            nc.sync.dma_start(w1_sb[:], w1_d[:].rearrange("k p c -> p k c"))
            nc.sync.dma_start(w2_sb[:], w2_d[:].rearrange("k p c -> p k c"))
            sw_sb = pp.tile([128, 2, VS], BF16, tag="sw")
            nc.sync.dma_start(sw_sb[:], sw_d[:].rearrange("k p c -> p k c"))
            swp_sb = pp.tile([128, 2, VS, 2], I16, tag="swp")
            nc.sync.dma_start(swp_sb[:],
                              swp_d[:].rearrange("k p c d -> p k c d"))
            hs0 = pp.tile([128, BT], BF16, tag="hs0")
            hs1 = pp.tile([128, BT], BF16, tag="hs1")

            ones_bf = pp.tile([128, 1], BF16, tag="ones")
            nc.gpsimd.memset(ones_bf[:], 1.0)
            half_sb = pp.tile([128, 1], F32, tag="half")
            nc.gpsimd.memset(half_sb[:], 0.5)

            c1 = pp.tile([32, H], F32, tag="c1")
            c2 = pp.tile([32, H], F32, tag="c2")
            nc.gpsimd.memset(c1[:], 0.0)
            nc.gpsimd.memset(c2[:], 0.0)

            se_sb = pp.tile([128, RT * NCHUNK], F32, tag="se")
            tg_sb = pp.tile([1, BT], F32, tag="tg")
            # accum_out adds into existing SBUF content on HW — zero it
            nc.gpsimd.memset(se_sb[:], 0.0)

            if has_b1:
                b1_sb = pp.tile([32, G4], F32, tag="b1")
                nc.sync.dma_start(b1_sb[:], b1_d[:])
            if has_b2:
                b2_sb = pp.tile([32, G4], F32, tag="b2")
                nc.sync.dma_start(b2_sb[:], b2_d[:])
            if has_swb:
                swb_sb = pp.tile([128, VS], F32, tag="swb")
                nc.sync.dma_start(swb_sb[:], swb_d[:])

            # ============ fused phase: gather + LSTM + logits ============
            with (
                tc.tile_pool(name="xsp", bufs=1) as xsp,
                tc.tile_pool(name="stage", bufs=3) as stp,
                tc.tile_pool(name="lwork", bufs=3) as lw,
                tc.tile_pool(name="zp", bufs=2, space="PSUM") as zp,
                tc.tile_pool(name="ep", bufs=3, space="PSUM") as ep,
                tc.tile_pool(name="ework", bufs=3) as ew,
            ):
                xs0 = xsp.tile([128, BT], BF16, tag="xs0")
                xs1 = xsp.tile([128, BT], BF16, tag="xs1")

                # ---- embedding gather (time-major) + transpose to slabs ----
                for rt in range(RT):
                    ids_sb = stp.tile([128, 1], I32, tag="ids")
                    nc.gpsimd.dma_start(ids_sb[:], ids_d.ap()[rt])
                    xrow = stp.tile([128, H], F32, tag="xrow")
                    nc.gpsimd.indirect_dma_start(
                        out=xrow[:], out_offset=None,
                        in_=emb_d[:],
                        in_offset=bass.IndirectOffsetOnAxis(
                            ap=ids_sb[:, :1], axis=0),
                    )
                    xbf = stp.tile([128, H], BF16, tag="xbf")
                    nc.vector.tensor_copy(xbf[:], xrow[:])
                    cs = 128 * rt
                    nc.sync.dma_start_transpose(
                        xs0[:, cs:cs + 128], xbf[:, 0:128])
                    nc.sync.dma_start_transpose(
                        xs1[:, cs:cs + 128], xbf[:, 128:256])

                def emit_logits_tile(rt):
                    cs = 128 * rt
                    for c0 in range(0, NCHUNK, 2):
                        cpair = [c for c in (c0, c0 + 1) if c < NCHUNK]
                        pses = []
                        for _c in cpair:
                            pse_t = ep.tile([128, CH], F32, tag="pse")
                            pses.append(pse_t)
                        for k in range(2):
                            hsk = hs0[:, cs:cs + 128] if k == 0 \
                                else hs1[:, cs:cs + 128]
                            for pse, c in zip(pses, cpair):
                                nc.tensor.matmul(
                                    pse[:], hsk,
                                    sw_sb[:, k, c * CH:c * CH + CH],
                                    start=(k == 0), stop=(k == 1),
                                )
                        for pse, c in zip(pses, cpair):
                            if has_swb:
                                nc.vector.tensor_tensor(
                                    out=pse[:], in0=pse[:],
                                    in1=swb_sb[:, c * CH:c * CH + CH],
                                    op=ALU.add)
                            ebuf = ew.tile([128, CH], BF16, tag="ebuf")
                            nc.scalar.activation(
                                ebuf[:], pse[:], AF.Exp,
                                accum_out=se_sb[:, rt * NCHUNK + c:
                                                rt * NCHUNK + c + 1])
                    # target logit for these 128 rows
                    tgi_sb = ew.tile([128, 8], I16, tag="tgi")
                    nc.gpsimd.dma_start(tgi_sb[:], tgi_d.ap()[rt])
                    pst = ep.tile([1, 128], F32, tag="pse")
                    for k in range(2):
                        swg = ew.tile([128, 128, 2], I16, tag="swg")
                        nc.gpsimd.ap_gather(
                            swg[:], swp_sb[:, k], tgi_sb[:],
                            channels=128, num_elems=VS, d=2, num_idxs=128,
                        )
                        mulk = ew.tile([128, 128], BF16, tag="mulk")
                        nc.vector.tensor_tensor(
                            out=mulk[:],
                            in0=swg[:].bitcast(BF16)[:, :, 0],
                            in1=hs0[:, cs:cs + 128] if k == 0
                            else hs1[:, cs:cs + 128],
                            op=ALU.mult)
                        nc.tensor.matmul(pst[:], ones_bf[:, 0:1], mulk[:],
                                         start=(k == 0), stop=(k == 1))
                    nc.scalar.copy(tg_sb[0:1, cs:cs + 128], pst[:])

                # ---- LSTM over T steps ----
                h1T_prev = None  # [128, 64] bf16 (k-tiles of h1^T)
                for t in range(T):
                    ts0 = 32 * t

                    def lstm_layer(lhsTs, w_sb, c_sb, bias_sb):
                        """One LSTM layer step. lhsTs: list of [128,32] bf16
                        k-tiles. Returns h_row [32, 256] bf16."""
                        psz = zp.tile([32, G4], F32, tag="z")
                        nk = len(lhsTs)
                        for k, lt in enumerate(lhsTs):
                            for nh in range(2):
                                nc.tensor.matmul(
                                    psz[:, 512 * nh:512 * nh + 512],
                                    lt,
                                    w_sb[:, k, 512 * nh:512 * nh + 512],
                                    start=(k == 0), stop=(k == nk - 1),
                                )
                        if bias_sb is not None:
                            nc.vector.tensor_tensor(
                                out=psz[:], in0=psz[:],
                                in1=bias_sb[:],
                                op=ALU.add)
                        # gates (host col order): i[0:256] o[256:512]
                        # f[512:768] j[768:1024]. sigmoid(x) is computed as
                        # 0.5*tanh(x/2)+0.5 (tanh+exp share one ACT table
                        # set, so LSTM and softmax-exp can interleave); the
                        # 0.5/0.5 affine folds into affine_mul_reduce.
                        g = lw.tile([32, G4], BF16, tag="g")
                        nc.scalar.activation(g[:, 0:512], psz[:, 0:512],
                                             AF.Tanh, scale=0.5)
                        nc.scalar.activation(g[:, 512:768], psz[:, 512:768],
                                             AF.Tanh, bias=half_sb[0:32, :1],
                                             scale=0.5)
                        nc.scalar.activation(g[:, 768:1024], psz[:, 768:1024],
                                             AF.Tanh)
                        # c = sig(f+1)*c + sig(i)*j ; h = tanh(c)*sig(o)
                        junk = lw.tile([32, 1], F32, tag="junk")
                        t1 = lw.tile([32, H], BF16, tag="t1")
                        nc.vector.affine_mul_reduce(
                            t1[:], junk[:], g[:, 0:256], g[:, 768:1024],
                            0.5, 0.5)
                        cf = lw.tile([32, H], F32, tag="cf")
                        nc.vector.affine_mul_reduce(
                            cf[:], junk[:], g[:, 512:768], c_sb[:], 0.5, 0.5)
                        nc.vector.tensor_tensor(out=c_sb[:], in0=cf[:],
                                                in1=t1[:], op=ALU.add)
                        tc_t = lw.tile([32, H], BF16, tag="tc")
                        nc.scalar.activation(tc_t[:], c_sb[:], AF.Tanh)
                        hrow = lw.tile([32, H], BF16, tag="hrow")
                        nc.vector.affine_mul_reduce(
                            hrow[:], junk[:], g[:, 256:512], tc_t[:],
                            0.5, 0.5)
                        return hrow

                    # layer 1: x k-tiles + h1 k-tiles
                    lhsTs = [xs0[:, ts0:ts0 + 32], xs1[:, ts0:ts0 + 32]]
                    if h1T_prev is not None:
                        lhsTs += [h1T_prev[:, 0:32], h1T_prev[:, 32:64]]
                    h1row = lstm_layer(lhsTs, w1_sb, c1,
                                       b1_sb if has_b1 else None)
                    # transpose h1 -> hidden-major k-tiles via the DMA xbar
                    # (keeps TensorE free)
                    h1T = lw.tile([128, 64], BF16, tag="h1T")
                    nc.sync.dma_start_transpose(h1T[:, 0:32], h1row[:, 0:128])
                    nc.sync.dma_start_transpose(h1T[:, 32:64],
                                                h1row[:, 128:256])
                    h1T_prev = h1T

                    # layer 2: h1 k-tiles + h2 k-tiles (prev step)
                    lhsTs = [h1T[:, 0:32], h1T[:, 32:64]]
                    if t > 0:
                        tp = 32 * (t - 1)
                        lhsTs += [hs0[:, tp:tp + 32], hs1[:, tp:tp + 32]]
                    h2row = lstm_layer(lhsTs, w2_sb, c2,
                                       b2_sb if has_b2 else None)
                    nc.sync.dma_start_transpose(hs0[:, ts0:ts0 + 32],
                                                h2row[:, 0:128])
                    nc.sync.dma_start_transpose(hs1[:, ts0:ts0 + 32],
                                                h2row[:, 128:256])

                    # interleave the logits/softmax tile for rows that just
                    # completed (keeps TensorE dense so HAM stays warm)
                    if t % 4 == 3:
                        emit_logits_tile(t // 4)

            nc.sync.dma_start(se_d[:], se_sb[:])
            nc.sync.dma_start(tg_d[:], tg_sb[:])

    nc.compile()
    meta = dict(T=T, V=V, n_cores=n_cores, B=B, H=H, VS=VS, BT=BT, RT=RT,
                CH=CH, NCHUNK=NCHUNK)
    return nc, meta


# ---------------- host-side prep / combine ----------------

def prep_inputs(meta, input_data, targets, embedding, W1, b1, W2, b2,
                softmax_w, softmax_b):
    """Build the per-core input maps (numpy)."""
    B, T, V = meta["B"], meta["T"], meta["V"]
    VS, RT, n_cores = meta["VS"], meta["RT"], meta["n_cores"]
    H = meta["H"]
    G4 = 4 * H

    ids_tm = np.ascontiguousarray(
        np.asarray(input_data, np.int64).T).reshape(-1)
    tgt_tm = np.ascontiguousarray(
        np.asarray(targets, np.int64).T).reshape(-1)
    ids_in = ids_tm.astype(np.int32).reshape(RT, 128, 1)

    # W column permutation [i, j, f, o] (TF order) -> [i, o, f, j]
    perm = np.concatenate([
        np.arange(0, H), np.arange(3 * H, 4 * H),
        np.arange(2 * H, 3 * H), np.arange(H, 2 * H)])

    def prep_w(W):
        Wp = W[:, perm].astype(ml_dtypes.bfloat16)          # [512, 1024]
        return np.ascontiguousarray(Wp.reshape(4, 128, G4))

    w1_in = prep_w(np.asarray(W1, np.float32))
    w2_in = prep_w(np.asarray(W2, np.float32))
    b1p = np.tile(np.asarray(b1, np.float32)[perm].reshape(1, G4), (32, 1))
    b2p = np.tile(np.asarray(b2, np.float32)[perm].reshape(1, G4), (32, 1))

    sw = np.asarray(softmax_w, np.float32)                  # [H, V]
    swb = np.asarray(softmax_b, np.float32)

    # vectorized ap_gather index layout: idx i lives at partition i%16,
    # column i//16, replicated per 16-partition group
    rtA = (np.arange(RT) * 128)[:, None, None]
    pA = (np.arange(128) % 16)[None, :, None]
    qA = (np.arange(8) * 16)[None, None, :]
    gat = rtA + qA + pA                                     # [RT, 128, 8]

    maps, masks = [], []
    for c in range(n_cores):
        shard = sw[:, c * VS:(c + 1) * VS].astype(ml_dtypes.bfloat16)
        sw_in = np.ascontiguousarray(shard.reshape(2, 128, VS))
        swi = sw_in.view(np.int16)
        swp_in = np.ascontiguousarray(
            np.stack([swi, swi], axis=-1))                  # [2,128,VS,2]

        tl = tgt_tm - c * VS
        inr = (tl >= 0) & (tl < VS)
        tlc = np.where(inr, tl, 0).astype(np.int16)
        tgi = tlc[gat]                                      # [RT, 128, 8]
        m = dict(ids=ids_in, emb=np.asarray(embedding, np.float32),
                 w1=w1_in, w2=w2_in, sw=sw_in, swp=swp_in, tgi=tgi)
        if np.any(b1p):
            m["b1p"] = b1p
        if np.any(b2p):
            m["b2p"] = b2p
        if np.any(swb):
            m["swbp"] = np.ascontiguousarray(
                np.tile(swb[c * VS:(c + 1) * VS].reshape(1, VS), (128, 1)))
        maps.append(m)
        masks.append(inr.astype(np.float32))
    return maps, masks, ids_tm, tgt_tm


def combine_outputs(meta, results, masks, tgt_tm, softmax_b):
    """results: list of per-core dicts with se_out [128, RT*NCHUNK] and
    tg_out [1, BT]. Returns the scalar cost (np.float32)."""
    B, T, BT = meta["B"], meta["T"], meta["BT"]
    RT, NCHUNK = meta["RT"], meta["NCHUNK"]
    se_all = np.zeros(BT, np.float64)
    tg_all = np.zeros(BT, np.float64)
    for c, r in enumerate(results):
        se = np.asarray(r["se_out"], np.float64)  # [128, RT*NCHUNK]
        se = se.reshape(128, RT, NCHUNK).sum(-1)  # [128, RT]
        se_all += se.T.reshape(-1)                # row r = rt*128 + p
        tg_all += np.asarray(r["tg_out"], np.float64)[0] * masks[c]
    tg_all += np.asarray(softmax_b, np.float64)[tgt_tm]
    loss = np.log(se_all) - tg_all
    return np.float32(loss.sum() / B / T)


# ---------------- public entry point ----------------

_CACHE = {}
last_exec_time_ns = None
last_trace_path = None


def _get_built(has_b1, has_b2, has_swb):
    key = (has_b1, has_b2, has_swb)
    if key not in _CACHE:
        _CACHE[key] = build_charrnn(T=T, V=V, n_cores=NCORES,
                                    has_b1=has_b1, has_b2=has_b2,
                                    has_swb=has_swb, num_devices=NCORES)
    return _CACHE[key]


def kernel(input_data, targets, embedding, W1, b1, W2, b2,
           softmax_w, softmax_b, _trace=False):
    global last_exec_time_ns, last_trace_path
    has_b1 = bool(np.any(np.asarray(b1)))
    has_b2 = bool(np.any(np.asarray(b2)))
    has_swb = bool(np.any(np.asarray(softmax_b)))
    nc, meta = _get_built(has_b1, has_b2, has_swb)
    maps, masks, ids_tm, tgt_tm = prep_inputs(
        meta, input_data, targets, embedding, W1, b1, W2, b2,
        softmax_w, softmax_b)
    res = run_bass_kernel_spmd(nc, maps, core_ids=list(range(NCORES)),
                               trace=_trace)
    last_exec_time_ns = res.exec_time_ns
    if res.instructions_and_trace is not None:
        last_trace_path = res.instructions_and_trace[1]
    cost = combine_outputs(meta, res.results, masks, tgt_tm, softmax_b)
    return np.asarray(cost, np.float32)



# revision 6
# speedup vs baseline: 2.2100x; 2.2100x over previous
"""Self-contained Trainium2 Bass kernel for the CharRNN problem:
2-layer LSTM (B=32, T=256, H=256) + V=32000 softmax cross-entropy mean loss.

Strategy (8 NeuronCores, SPMD):
  * LSTM recurrence replicated on every core (latency-bound); softmax
    sharded over vocab (VS=4000/core); host combines partial sums.
  * Per-step gates are computed with COLUMN-TILED matmuls
    (tile_position=(0,32j)): partition strip j (rows 32j:32j+32) holds
    batch rows for HIDDEN QUARTER j, with the strip's 256 psum columns
    = [i|o|f|jnew] x 64 units. The 4 strip matmuls stream concurrently
    on the PE sub-arrays, and the gate nonlinearity becomes ONE
    128-partition Tanh ACT instr (sigmoid = 0.5*tanh(x/2)+0.5 with the
    0.5 pre-scaled into W, forget bias injected via a K=1 ones-row
    matmul in the accumulation group).
  * All elementwise state math is [128, 64] (batch x quarter
    interleaved); h is transposed back to hidden-major k-tiles with 4
    tiny PE transposes (tile_position row/col placement) + 1 DVE copy
    instead of 1.2us DMA transposes.
  * Softmax: logits matmuls into 2-bank psum pairs, exp via wide ACT
    instrs with accum_out; per-row target logit via gpsimd ap_gather
    (int16-pair view of sw) + multiply + ones-matmul reduce.
  * Host combines: loss_r = log(sum_cores se_r) - tgt_logit_r.
"""
import os
import numpy as np
import ml_dtypes
import concourse.bass as bass
import concourse.mybir as mybir
import concourse.tile as tile
from concourse import bacc
from concourse.masks import make_identity
from concourse.bass_utils import run_bass_kernel_spmd

F32 = mybir.dt.float32
BF16 = mybir.dt.bfloat16
I32 = mybir.dt.int32
I16 = mybir.dt.int16
AF = mybir.ActivationFunctionType
ALU = mybir.AluOpType

B, T, H, V, NCORES = 32, 256, 256, 32000, 8


def build_charrnn(T=256, V=32000, n_cores=8, has_swb=False, num_devices=8):
    B, H = 32, 256
    G4 = 4 * H
    VS = V // n_cores
    BT = B * T
    RT = BT // 128                  # 128-row tiles (4 steps each)
    assert T % 4 == 0 and BT % 128 == 0

    CH = 500                        # logits chunk (<=512 = one psum bank)
    NCHUNK = VS // CH               # 8 chunks per tile
    assert VS % CH == 0 and NCHUNK % 2 == 0
    NEXP = NCHUNK // 2              # exp instrs per tile (2 chunks each)

    nc = bacc.Bacc("TRN2", target_bir_lowering=False, debug=False,
                   num_devices=num_devices)

    # ---------------- DRAM I/O ----------------
    ids_d = nc.dram_tensor("ids", (RT, 128, 1), I32, kind="ExternalInput")
    emb_d = nc.dram_tensor("emb", (V, H), BF16, kind="ExternalInput")
    w1_d = nc.dram_tensor("w1", (4, 128, G4), BF16, kind="ExternalInput")
    w2_d = nc.dram_tensor("w2", (4, 128, G4), BF16, kind="ExternalInput")
    br1_d = nc.dram_tensor("brow1", (1, G4), BF16, kind="ExternalInput")
    br2_d = nc.dram_tensor("brow2", (1, G4), BF16, kind="ExternalInput")
    sw_d = nc.dram_tensor("sw", (2, 128, VS), BF16, kind="ExternalInput")
    swp_d = nc.dram_tensor("swp", (2, 128, VS, 2), I16, kind="ExternalInput")
    tgi_d = nc.dram_tensor("tgi", (RT, 128, 8), I16, kind="ExternalInput")
    if has_swb:
        swb_d = nc.dram_tensor("swbp", (128, VS), F32, kind="ExternalInput")
    se_d = nc.dram_tensor("se_out", (128, RT * NEXP), F32,
                          kind="ExternalOutput")
    tg_d = nc.dram_tensor("tg_out", (1, BT), F32, kind="ExternalOutput")

    with tile.TileContext(nc) as tc:
        with tc.tile_pool(name="persist", bufs=1) as pp:
            # ---- persistent SBUF ----
            w1_sb = pp.tile([128, 4, G4], BF16, tag="w1")
            w2_sb = pp.tile([128, 4, G4], BF16, tag="w2")
            nc.sync.dma_start(w1_sb[:], w1_d[:].rearrange("k p c -> p k c"))
            nc.sync.dma_start(w2_sb[:], w2_d[:].rearrange("k p c -> p k c"))
            br1 = pp.tile([1, G4], BF16, tag="br1")
            br2 = pp.tile([1, G4], BF16, tag="br2")
            nc.sync.dma_start(br1[:], br1_d[:])
            nc.sync.dma_start(br2[:], br2_d[:])
            sw_sb = pp.tile([128, 2, VS], BF16, tag="sw")
            nc.sync.dma_start(sw_sb[:], sw_d[:].rearrange("k p c -> p k c"))
            swp_sb = pp.tile([128, 2, VS, 2], I16, tag="swp")
            nc.sync.dma_start(swp_sb[:],
                              swp_d[:].rearrange("k p c d -> p k c d"))
            if has_swb:
                swb_sb = pp.tile([128, VS], F32, tag="swb")
                nc.sync.dma_start(swb_sb[:], swb_d[:])

            xsT = pp.tile([128, 2, BT], BF16, tag="xsT")
            hsT = pp.tile([128, 2, BT], BF16, tag="hsT")

            ones1 = pp.tile([1, 32], BF16, tag="ones1")
            nc.gpsimd.memset(ones1[:], 1.0)
            onesc = pp.tile([128, 1], BF16, tag="onesc")
            nc.gpsimd.memset(onesc[:], 1.0)
            ident = pp.tile([128, 128], BF16, tag="ident")
            make_identity(nc, ident[:])

            c1 = pp.tile([128, 64], F32, tag="c1")
            c2 = pp.tile([128, 64], F32, tag="c2")
            nc.gpsimd.memset(c1[:], 0.0)
            nc.gpsimd.memset(c2[:], 0.0)
            junk = pp.tile([128, 1], F32, tag="junk")

            h1T = pp.tile([128, 64], BF16, tag="h1T")

            se_sb = pp.tile([128, RT * NEXP], F32, tag="se")
            tg_sb = pp.tile([1, BT], F32, tag="tg")
            # accum_out adds into existing SBUF content on HW — zero it
            nc.gpsimd.memset(se_sb[:], 0.0)

            with (
                tc.tile_pool(name="stage", bufs=3) as stp,
                tc.tile_pool(name="gwork", bufs=2) as gw,
                tc.tile_pool(name="lwork", bufs=2) as lw,
                tc.tile_pool(name="z1p", bufs=1, space="PSUM") as z1p,
                tc.tile_pool(name="z2p", bufs=1, space="PSUM") as z2p,
                tc.tile_pool(name="lgp", bufs=2, space="PSUM") as lgp,
                tc.tile_pool(name="htp", bufs=1, space="PSUM") as htp,
                tc.tile_pool(name="ptp", bufs=1, space="PSUM") as ptp,
                tc.tile_pool(name="ew", bufs=3) as ew,
            ):
                # ---- embedding gather (time-major) + transpose to slabs ----
                for rt in range(RT):
                    ids_sb = stp.tile([128, 1], I32, tag="ids")
                    nc.gpsimd.dma_start(ids_sb[:], ids_d.ap()[rt])
                    xrow = stp.tile([128, H], BF16, tag="xrow")
                    nc.gpsimd.indirect_dma_start(
                        out=xrow[:], out_offset=None,
                        in_=emb_d[:],
                        in_offset=bass.IndirectOffsetOnAxis(
                            ap=ids_sb[:, :1], axis=0),
                    )
                    cs = 128 * rt
                    nc.sync.dma_start_transpose(
                        xsT[:, 0, cs:cs + 128], xrow[:, 0:128])
                    nc.sync.dma_start_transpose(
                        xsT[:, 1, cs:cs + 128], xrow[:, 128:256])

                def emit_pair(rt, s):
                    """Logits matmuls for chunk-pair s of row-tile rt.
                    Returns a closure that emits the exp (run ~1 step
                    later so the ACT never waits on these matmuls)."""
                    cs = 128 * rt
                    lg = lgp.tile([128, 2, 512], F32, tag="lg")
                    for half in range(2):
                        ch = s * 2 + half
                        for k in range(2):
                            nc.tensor.matmul(
                                lg[:, half, 0:CH],
                                hsT[:, k, cs:cs + 128],
                                sw_sb[:, k, ch * CH:ch * CH + CH],
                                start=(k == 0), stop=(k == 1),
                            )

                    def do_exp():
                        if has_swb:
                            for half in range(2):
                                ch = s * 2 + half
                                nc.vector.tensor_tensor(
                                    out=lg[:, half, 0:CH],
                                    in0=lg[:, half, 0:CH],
                                    in1=swb_sb[:, ch * CH:ch * CH + CH],
                                    op=ALU.add)
                        ebuf = ew.tile([128, 2, CH], BF16, tag="ebuf")
                        col = rt * NEXP + s
                        nc.scalar.activation(
                            ebuf[:], lg[:, :, 0:CH], AF.Exp,
                            accum_out=se_sb[:, col:col + 1])
                    return do_exp

                def emit_tgt(rt):
                    # target logit for row-tile rt's 128 rows
                    cs = 128 * rt
                    tgi_sb = ew.tile([128, 8], I16, tag="tgi")
                    nc.gpsimd.dma_start(tgi_sb[:], tgi_d.ap()[rt])
                    pst = ptp.tile([1, 128], F32, tag="pst")
                    for k in range(2):
                        swg = ew.tile([128, 128, 2], I16, tag="swg")
                        nc.gpsimd.ap_gather(
                            swg[:], swp_sb[:, k], tgi_sb[:],
                            channels=128, num_elems=VS, d=2, num_idxs=128,
                        )
                        mulk = ew.tile([128, 128], BF16, tag="mulk")
                        nc.vector.tensor_tensor(
                            out=mulk[:],
                            in0=swg[:].bitcast(BF16)[:, :, 0],
                            in1=hsT[:, k, cs:cs + 128],
                            op=ALU.mult)
                        nc.tensor.matmul(pst[:], onesc[:, 0:1], mulk[:],
                                         start=(k == 0), stop=(k == 1))
                    nc.scalar.copy(tg_sb[0:1, cs:cs + 128], pst[:])

                def lstm_layer(zpool, w_sb, brow, c_sb, xks, hks, hTdst):
                    """One layer step. xks/hks: list of (lhsT k-tile AP,
                    k index). hTdst: psum AP [128, 64] for the h
                    transposes. Returns h_int [128, 64] bf16."""
                    z = zpool.tile([128, 256], F32, tag="z")
                    nk = len(xks) + len(hks)
                    for j in range(4):
                        nc.tensor.matmul(
                            z[32 * j:32 * j + 32, :], ones1[0:1, :],
                            brow[0:1, 256 * j:256 * j + 256],
                            start=True, stop=False,
                            tile_position=(0, 32 * j), skip_group_check=True)
                    for i, (lt, k) in enumerate(xks + hks):
                        last = (i == nk - 1)
                        for j in range(4):
                            nc.tensor.matmul(
                                z[32 * j:32 * j + 32, :], lt,
                                w_sb[:, k, 256 * j:256 * j + 256],
                                start=False, stop=last,
                                tile_position=(0, 32 * j),
                                skip_group_check=True)
                    # one Tanh for all gates: cols [i|o|f|j] x 64
                    g = gw.tile([128, 256], BF16, tag="g")
                    nc.scalar.activation(g[:], z[:], AF.Tanh)
                    # c = sig(f+1)*c + sig(i)*tanh(j); h = tanh(c)*sig(o)
                    u = lw.tile([128, 64], BF16, tag="u")
                    nc.vector.affine_mul_reduce(
                        u[:], junk[:], g[:, 0:64], g[:, 192:256], 0.5, 0.5)
                    v = lw.tile([128, 64], F32, tag="v")
                    nc.vector.affine_mul_reduce(
                        v[:], junk[:], g[:, 128:192], c_sb[:], 0.5, 0.5)
                    nc.vector.tensor_tensor(out=c_sb[:], in0=u[:], in1=v[:],
                                            op=ALU.add)
                    tc_t = lw.tile([128, 64], BF16, tag="tc")
                    nc.scalar.activation(tc_t[:], c_sb[:], AF.Tanh)
                    hrow = lw.tile([128, 64], BF16, tag="hrow")
                    nc.vector.affine_mul_reduce(
                        hrow[:], junk[:], g[:, 64:128], tc_t[:], 0.5, 0.5)
                    # transpose to hidden-major k-tiles:
                    # quarter j -> [64, 32] at (partition 64*(j%2), col 32*(j//2))
                    for j in range(4):
                        pj = 64 * (j % 2)
                        cj = 32 * (j // 2)
                        nc.tensor.transpose(
                            hTdst[pj:pj + 64, cj:cj + 32],
                            hrow[32 * j:32 * j + 32, :],
                            ident[32 * j:32 * j + 32, 32 * j:32 * j + 32],
                            tile_position=(32 * j, pj),
                        )
                    return hrow

                # ---- LSTM over T steps ----
                exp_q = []
                for t in range(T):
                    ts0 = 32 * t
                    hpair = htp.tile([128, 128], BF16, tag="hpair")
                    # layer 1: x k-tiles (k=0,1) + h1 k-tiles (k=2,3)
                    xks = [(xsT[:, k, ts0:ts0 + 32], k) for k in range(2)]
                    hks = [] if t == 0 else \
                        [(h1T[:, 32 * k:32 * k + 32], 2 + k)
                         for k in range(2)]
                    lstm_layer(z1p, w1_sb, br1, c1, xks, hks, hpair[:, 0:64])
                    nc.vector.tensor_copy(h1T[:], hpair[:, 0:64])

                    # layer 2: h2 k-tiles (prev step, k=2,3) + h1 (k=0,1)
                    hks2 = [] if t == 0 else \
                        [(hsT[:, k, ts0 - 32:ts0], 2 + k) for k in range(2)]
                    xks2 = [(h1T[:, 32 * k:32 * k + 32], k) for k in range(2)]
                    lstm_layer(z2p, w2_sb, br2, c2, hks2, xks2,
                               hpair[:, 64:128])
                    nc.vector.tensor_copy(hsT[:, 0:2, ts0:ts0 + 32],
                                          hpair[:, 64:128])

                    # softmax: one chunk-pair of tile rt per step, spread
                    # over steps 4rt+3 .. 4rt+6; exp runs one step later
                    if t >= 3:
                        rt, s = divmod(t - 3, 4)
                        if s == 0:
                            emit_tgt(rt)
                        exp_q.append(emit_pair(rt, s))
                    while len(exp_q) > 1:
                        exp_q.pop(0)()

                for tt in range(T, T + 3):
                    rt, s = divmod(tt - 3, 4)
                    if s == 0:
                        emit_tgt(rt)
                    exp_q.append(emit_pair(rt, s))
                while exp_q:
                    exp_q.pop(0)()

            nc.sync.dma_start(se_d[:], se_sb[:])
            nc.sync.dma_start(tg_d[:], tg_sb[:])

    nc.compile()
    meta = dict(T=T, V=V, n_cores=n_cores, B=B, H=H, VS=VS, BT=BT, RT=RT,
                CH=CH, NCHUNK=NCHUNK, NEXP=NEXP)
    return nc, meta


# ---------------- host-side prep / combine ----------------

def prep_inputs(meta, input_data, targets, embedding, W1, b1, W2, b2,
                softmax_w, softmax_b):
    """Build the per-core input maps (numpy)."""
    B, T, V = meta["B"], meta["T"], meta["V"]
    VS, RT, n_cores = meta["VS"], meta["RT"], meta["n_cores"]
    H = meta["H"]
    G4 = 4 * H

    ids_tm = np.ascontiguousarray(
        np.asarray(input_data, np.int64).T).reshape(-1)
    tgt_tm = np.ascontiguousarray(
        np.asarray(targets, np.int64).T).reshape(-1)
    ids_in = ids_tm.astype(np.int32).reshape(RT, 128, 1)

    # W column permutation: new col = 256*jq + 64*g + u  <-  tf col
    # tfblock(g)*256 + 64*jq + u, g order [i,o,f,jnew] -> tf [i,j,f,o]
    tfblock = [0, 3, 2, 1]
    jq = np.arange(G4) // 256
    g = (np.arange(G4) % 256) // 64
    u = np.arange(G4) % 64
    perm = np.array(tfblock)[g] * 256 + 64 * jq + u
    scale = np.where(g < 3, 0.5, 1.0).astype(np.float32)  # i,o,f sigmoid

    def prep_w(W):
        Wp = (np.asarray(W, np.float32)[:, perm] * scale[None, :]).astype(
            ml_dtypes.bfloat16)
        return np.ascontiguousarray(Wp.reshape(4, 128, G4))

    def prep_b(b):
        bp = np.asarray(b, np.float32)[perm] * scale
        bp = bp + np.where(g == 2, 0.5, 0.0)      # forget bias (scaled)
        return np.ascontiguousarray(
            bp.astype(ml_dtypes.bfloat16).reshape(1, G4))

    w1_in = prep_w(W1)
    w2_in = prep_w(W2)
    br1 = prep_b(b1)
    br2 = prep_b(b2)

    emb_in = np.ascontiguousarray(
        np.asarray(embedding, np.float32).astype(ml_dtypes.bfloat16))

    sw = np.asarray(softmax_w, np.float32)                  # [H, V]
    swb = np.asarray(softmax_b, np.float32)
    has_swb = bool(np.any(swb))

    # vectorized ap_gather index layout: idx i lives at partition i%16,
    # column i//16, replicated per 16-partition group
    rtA = (np.arange(RT) * 128)[:, None, None]
    pA = (np.arange(128) % 16)[None, :, None]
    qA = (np.arange(8) * 16)[None, None, :]
    gat = rtA + qA + pA                                     # [RT, 128, 8]

    maps, masks = [], []
    for c in range(n_cores):
        shard = sw[:, c * VS:(c + 1) * VS].astype(ml_dtypes.bfloat16)
        sw_in = np.ascontiguousarray(shard.reshape(2, 128, VS))
        swi = sw_in.view(np.int16)
        swp_in = np.ascontiguousarray(
            np.stack([swi, swi], axis=-1))                  # [2,128,VS,2]

        tl = tgt_tm - c * VS
        inr = (tl >= 0) & (tl < VS)
        tlc = np.where(inr, tl, 0).astype(np.int16)
        tgi = tlc[gat]                                      # [RT, 128, 8]
        m = dict(ids=ids_in, emb=emb_in,
                 w1=w1_in, w2=w2_in, brow1=br1, brow2=br2,
                 sw=sw_in, swp=swp_in, tgi=tgi)
        if has_swb:
            m["swbp"] = np.ascontiguousarray(
                np.tile(swb[c * VS:(c + 1) * VS].reshape(1, VS), (128, 1)))
        maps.append(m)
        masks.append(inr.astype(np.float32))
    return maps, masks, ids_tm, tgt_tm, has_swb


def combine_outputs(meta, results, masks, tgt_tm, softmax_b):
    """results: list of per-core dicts with se_out [128, RT*NEXP] and
    tg_out [1, BT]. Returns the scalar cost (np.float32)."""
    B, T, BT = meta["B"], meta["T"], meta["BT"]
    RT, NEXP = meta["RT"], meta["NEXP"]
    se_all = np.zeros(BT, np.float64)
    tg_all = np.zeros(BT, np.float64)
    for c, r in enumerate(results):
        se = np.asarray(r["se_out"], np.float64)  # [128, RT*NEXP]
        se = se.reshape(128, RT, NEXP).sum(-1)    # [128, RT]
        se_all += se.T.reshape(-1)                # row r = rt*128 + p
        tg_all += np.asarray(r["tg_out"], np.float64)[0] * masks[c]
    tg_all += np.asarray(softmax_b, np.float64)[tgt_tm]
    loss = np.log(se_all) - tg_all
    return np.float32(loss.sum() / B / T)


# ---------------- public entry point ----------------

_CACHE = {}
last_exec_time_ns = None
last_trace_path = None


def _get_built(T_, has_swb):
    key = (T_, has_swb)
    if key not in _CACHE:
        _CACHE[key] = build_charrnn(T=T_, V=V, n_cores=NCORES,
                                    has_swb=has_swb, num_devices=NCORES)
    return _CACHE[key]


def kernel(input_data, targets, embedding, W1, b1, W2, b2,
           softmax_w, softmax_b, _trace=False):
    global last_exec_time_ns, last_trace_path
    T_ = int(np.asarray(input_data).shape[1])
    has_swb = bool(np.any(np.asarray(softmax_b)))
    nc, meta = _get_built(T_, has_swb)
    maps, masks, ids_tm, tgt_tm, _ = prep_inputs(
        meta, input_data, targets, embedding, W1, b1, W2, b2,
        softmax_w, softmax_b)
    res = run_bass_kernel_spmd(nc, maps, core_ids=list(range(NCORES)),
                               trace=_trace)
    last_exec_time_ns = res.exec_time_ns
    if res.instructions_and_trace is not None:
        last_trace_path = res.instructions_and_trace[1]
    cost = combine_outputs(meta, res.results, masks, tgt_tm, softmax_b)
    return np.asarray(cost, np.float32)
